# revision 12
# baseline (speedup 1.0000x reference)
"""Trainium2 Bass kernel for nn_Attention_20933670601301.

Math (per batch b, with P[b] in [n, C], n=512, C=256):
    p_sel = P[b, id[b]]                     # [C]
    qk    = Wk^T (Wq p_sel + bq) = M p_sel + v    (M, v folded on host)
    scores= P[b] @ qk  (+ const)            # [n]; const cancels in softmax
    attn  = softmax(scores)
    out   = Wv @ (P[b]^T attn) + bv         # sum(attn)==1 absorbs bk term

Layout strategy: the host ships P TRANSPOSED per batch (PT[b] = P[b]^T,
fp16, 8 MiB/core — the only big stream).  Both big contractions then run
on the PE as stationary-weight matmuls (cheap: cost scales with the
moving free size, which is 1):
  scores column: lhsT = PT chunk [c-part, n-cols], rhs = qk col  -> [n,1]
  t^T    column: lhsT = P  chunk [n-part, c-cols], rhs = attn col -> [c,1]
The natural-layout P needed by the t-stage is regenerated on-chip with
PE transposes (fp16 -> fp16 PSUM) whose PSUM->SBUF copies are split
across DVE/ACT/Pool so no single slow engine becomes the bottleneck.
Softmax runs batched per group of 8 in fp32 exactly as before.

Data-parallel across 8 cores on the batch dim; weights replicated,
fused + pre-transposed on the host to match the PE's lhsT layout.
"""

import numpy as np

B, N, C = 256, 512, 256
NCORES = 8
BL = B // NCORES      # 32 batches per core
NK = N // 128         # 4 chunks of 128 rows
G = 16                # softmax group size (batches): engine cost per group
                      # is free-size-bound (independent of G), so fewer,
                      # larger groups mean less total softmax work
NG = BL // G

_CACHE = {}


def _build():
    from contextlib import ExitStack

    import concourse.bass as bass
    import concourse.mybir as mybir
    import concourse.tile as tile
    from concourse import bacc
    from concourse.masks import make_identity

    dt = mybir.dt
    AF = mybir.ActivationFunctionType
    OP = mybir.AluOpType
    f32, f16 = dt.float32, dt.float16

    nc = bacc.Bacc("TRN2", target_bir_lowering=False)
    pt_d = nc.dram_tensor("pt", [BL, C, N], f16, kind="ExternalInput")
    # p_sel = P[b, id[b]] is gathered host-side: the device indirect-DMA path
    # (DynamicDMA) is disabled in this walrus build and hard-faults the NRT
    psel_d = nc.dram_tensor("psel", [BL, C], f32, kind="ExternalInput")
    # packed host-side: w = [M^T, Wv^T], b = [v, bv]  (M = Wk^T Wq, v = Wk^T bq)
    w_d = nc.dram_tensor("w", [2, C, C], f32, kind="ExternalInput")
    b_d = nc.dram_tensor("b", [2, C], f32, kind="ExternalInput")
    bvn_d = nc.dram_tensor("bvn", [G, C], f32, kind="ExternalInput")
    out_d = nc.dram_tensor("out", [BL, C], f32, kind="ExternalOutput")

    with tile.TileContext(nc) as tc, ExitStack() as ctx:
        consts = ctx.enter_context(tc.tile_pool(name="consts", bufs=1))
        big = ctx.enter_context(tc.tile_pool(name="big", bufs=1))
        sgrp = ctx.enter_context(tc.tile_pool(name="sgrp", bufs=2))
        onat = ctx.enter_context(tc.tile_pool(name="onat", bufs=2))
        # PSUM banks: ptp 3 + scg 1 + ptt 2x2 + psm 2 = 8 of 8
        # (pools allocate bufs slots per tag, bank-granular)
        ptp = ctx.enter_context(tc.tile_pool(name="ptp", bufs=3, space="PSUM"))
        scT = ctx.enter_context(tc.tile_pool(name="scT", bufs=1, space="PSUM"))
        ptt = ctx.enter_context(tc.tile_pool(name="ptt", bufs=1, space="PSUM"))
        psm = ctx.enter_context(tc.tile_pool(name="psm", bufs=2, space="PSUM"))

        # ---- identities + early DMAs ----
        ident = consts.tile([128, 128], f32)
        make_identity(nc, ident)
        ident16 = consts.tile([128, 128], f16)
        make_identity(nc, ident16)
        p_sel = consts.tile([BL, C], f32)
        nc.sync.dma_start(out=p_sel, in_=psel_d[:, :])

        # ---- persistent state ----
        pt_sb = big.tile([128, BL, 2, N], f16)       # PT stream  [c-half part, b, h, n]
        p_nat = big.tile([128, BL, NK, C], f16)      # natural    [n part, b, k, c]
        scores_sb = consts.tile([128, NK, BL], f32)  # [n part, k, b]
        attnT = consts.tile([128, NK, BL], f16)
        tT_sb = consts.tile([128, 2, BL], f32)
        w_sb = consts.tile([128, 2, 2, 2, 128], f32)
        b_sb = consts.tile([128, 2, 2], f32)
        wm_sb = w_sb[:, 0]
        wvt_sb = w_sb[:, 1]
        v_sb = b_sb[:, 0]
        bv_sb = b_sb[:, 1]
        qkT_sb = consts.tile([128, 2, BL], f32)
        qk16 = consts.tile([128, 2, BL], f16)
        bvn_sb = consts.tile([G, 2, 128], f32)

        def load_weights():
            nc.scalar.dma_start(
                out=w_sb,
                in_=w_d[:, :, :].rearrange(
                    "w (kc kp) (mc mp) -> kp w kc mc mp", kp=128, mp=128
                ),
            )
            nc.scalar.dma_start(
                out=b_sb, in_=b_d[:, :].rearrange("w (c p) -> p w c", p=128)
            )
            nc.scalar.dma_start(
                out=bvn_sb, in_=bvn_d[:, :].rearrange("g (m c) -> g m c", m=2)
            )

        def setup_qk():
            # p_selT [C-part, b]
            p_selT = consts.tile([128, 2, BL], f32)
            for h in range(2):
                pst = psm.tile([128, BL], f32, tag="s")
                nc.tensor.transpose(
                    out=pst,
                    in_=p_sel[:, h * 128 : (h + 1) * 128],
                    identity=ident[:BL, :BL],
                )
                nc.vector.tensor_copy(out=p_selT[:, h, :], in_=pst)
            # qk^T = M @ p_sel^T + v  -> [C-part, b]
            qk_ps = psm.tile([128, 2, BL], f32, tag="s")
            for mc in range(2):
                for kc in range(2):
                    nc.tensor.matmul(
                        out=qk_ps[:, mc, :],
                        lhsT=wm_sb[:, kc, mc, :],
                        rhs=p_selT[:, kc, :],
                        start=(kc == 0),
                        stop=(kc == 1),
                    )
            for mc in range(2):
                nc.scalar.activation(
                    out=qkT_sb[:, mc, :],
                    in_=qk_ps[:, mc, :],
                    func=AF.Identity,
                    bias=v_sb[:, mc : mc + 1],
                    scale=1.0,
                )
            nc.vector.tensor_copy(out=qk16, in_=qkT_sb)

        def load_part(b0, nb):
            nc.sync.dma_start(
                out=pt_sb[:, b0 : b0 + nb],
                in_=pt_d[b0 : b0 + nb, :, :].rearrange(
                    "b (h p) n -> p b h n", p=128
                ),
            )

        # scores PSUM group tiles (accumulated across the group's batches)
        sc_ps = {}

        def scores_batch(b):
            g, j = divmod(b, G)
            if j == 0:
                sc_ps[g] = scT.tile([128, NK, G], f32, tag="scg", name="scg")
            t = sc_ps[g]
            for k in range(NK):
                for h in range(2):
                    nc.tensor.matmul(
                        out=t[:, k, j : j + 1],
                        lhsT=pt_sb[:, b, h, k * 128 : (k + 1) * 128],
                        rhs=qk16[:, h, b : b + 1],
                        start=(h == 0),
                        stop=(h == 1),
                    )

        # PSUM->SBUF copy engine per batch: GPSIMD cannot read PSUM, so the
        # copies split between DVE and ACT, balanced against their other work
        cp_eng = []
        acc = {"D": 0.0, "A": 0.0}
        rate = {"D": 0.66, "A": 1.04}  # us per copy
        base = {"D": 4.0, "A": 2.2}    # other busy work
        for _ in range(BL):
            e = min(acc, key=lambda k: base[k] + acc[k] + rate[k])
            acc[e] += rate[e]
            cp_eng.append(e)
        cp_eng[BL - 6 :] = ["D", "A", "D", "A", "D", "A"]

        def trans_batch(b):
            tp = ptp.tile([128, NK, 2, 128], f16, tag="ptp")
            for k in range(NK):
                for h in range(2):
                    nc.tensor.transpose(
                        out=tp[:, k, h, :],
                        in_=pt_sb[:, b, h, k * 128 : (k + 1) * 128],
                        identity=ident16,
                    )
            e = cp_eng[b]
            dst = p_nat[:, b, :, :].rearrange("p k (h c) -> p k h c", h=2)
            if e == "D":
                nc.vector.tensor_copy(out=dst, in_=tp)
            else:
                nc.scalar.copy(out=dst, in_=tp)

        # ---- softmax + t + out stages (per group) ----
        grp_state = {}

        def phase_b1(g):
            gs = slice(g * G, (g + 1) * G)
            nc.scalar.copy(out=scores_sb[:, :, gs], in_=sc_ps.pop(g))
            sp = psm.tile([G, NK, 128], f32, tag="s")
            for k in range(NK):
                nc.tensor.transpose(
                    out=sp[:, k, :],
                    in_=scores_sb[:, k, gs],
                    identity=ident,
                )
            grp_state[g] = sp

        def phase_b2a(g):
            # max/exp read the transposed scores straight from PSUM
            sc_nat = grp_state[g]
            negmax = sgrp.tile([G, 1], f32, tag="negmax")
            nc.vector.tensor_reduce(
                out=negmax,
                in_=sc_nat[:, :, :],
                axis=mybir.AxisListType.XY,
                op=OP.max,
                negate=True,
            )
            grp_state[g] = (sc_nat, negmax)

        def phase_b2b(g):
            sc_nat, negmax = grp_state[g]
            attn_nat = sgrp.tile([G, NK, 128], f32, tag="attnnat")
            esum = sgrp.tile([G, 1], f32, tag="esum")
            nc.scalar.activation(
                out=attn_nat,
                in_=sc_nat[:, :, :],
                func=AF.Exp,
                bias=negmax[:, :1],
                scale=1.0,
                accum_out=esum,
            )
            grp_state[g] = (attn_nat, esum)

        rs_of = {}

        def phase_b3(g):
            # attn stays UNNORMALIZED: 1/sum is applied to the final output
            # rows (fused scalar_tensor_tensor in phase_d), keeping the
            # normalize off the softmax->t critical chain entirely
            attn_nat, esum = grp_state.pop(g)
            rs = sgrp.tile([G, 1], f32, tag="rs")
            nc.vector.reciprocal(rs, esum)
            rs_of[g] = rs
            ap_ps = psm.tile([128, NK, G], f32, tag="s")
            for k in range(NK):
                nc.tensor.transpose(
                    out=ap_ps[:, k, :], in_=attn_nat[:, k, :], identity=ident[:G, :G]
                )
            nc.vector.tensor_copy(
                out=attnT[:, :, g * G : (g + 1) * G], in_=ap_ps
            )

        def phase_t(g):
            gs = slice(g * G, (g + 1) * G)
            tT_g = ptt.tile([128, 2, G], f32, tag="tTg")
            for j in range(G):
                b = g * G + j
                for h in range(2):
                    for k in range(NK):
                        nc.tensor.matmul(
                            out=tT_g[:, h, j : j + 1],
                            lhsT=p_nat[:, b, k, h * 128 : (h + 1) * 128],
                            rhs=attnT[:, k, b : b + 1],
                            start=(k == 0),
                            stop=(k == NK - 1),
                        )
            nc.scalar.copy(out=tT_sb[:, :, gs], in_=tT_g)

        def phase_d(g):
            gs = slice(g * G, (g + 1) * G)
            o_ps = psm.tile([128, 2, G], f32, tag="s")
            for mc in range(2):
                for kc in range(2):
                    nc.tensor.matmul(
                        out=o_ps[:, mc, :],
                        lhsT=wvt_sb[:, kc, mc, :],
                        rhs=tT_sb[:, kc, gs],
                        start=(kc == 0),
                        stop=(kc == 1),
                    )
            outT_g = sgrp.tile([128, 2, G], f32, tag="outT")
            nc.vector.tensor_copy(out=outT_g, in_=o_ps)
            out_nat = onat.tile([G, 2, 128], f32, tag="outnat")
            op_ps = ptt.tile([G, 2, 128], f32, tag="op")
            for mc in range(2):
                nc.tensor.transpose(
                    out=op_ps[:, mc, :], in_=outT_g[:, mc, :], identity=ident
                )
            nc.vector.scalar_tensor_tensor(
                out=out_nat,
                in0=op_ps,
                scalar=rs_of.pop(g)[:, :1],
                in1=bvn_sb,
                op0=OP.mult,
                op1=OP.add,
            )
            # SWDGE on the (mostly idle) Pool engine: the SP queue must stay
            # clear for input loads — an out DMA there blocks them at the head
            nc.gpsimd.dma_start(out=out_d[gs, :], in_=out_nat[:, :, :])

        # ---- schedule ----
        chunks = [(0, 1), (1, 1)] + [(b0, 2) for b0 in range(2, BL, 2)]
        stages = [
            (phase_b1, 0),
            (phase_b2a, 2),
            (phase_b2b, 4),
            (lambda g: (phase_b3(g), phase_t(g)), 6),
            (phase_d, 8),
        ]
        nstage = [0] * len(stages)

        def run_stages(done_a):
            for si, (fn, off) in enumerate(stages):
                lim = nstage[si - 1] if si else NG
                while nstage[si] < lim and done_a >= nstage[si] * G + G + off:
                    fn(nstage[si])
                    nstage[si] += 1

        # first two single-batch loads are emitted before the weight DMAs;
        # their compute is emitted only AFTER setup_qk has written qk16
        # (Tile tracks dependencies in emission order).
        todo_trans = []
        for b0, nb in chunks[:2]:
            load_part(b0, nb)
        load_weights()
        setup_qk()
        for b0, nb in chunks[:2]:
            for b in range(b0, b0 + nb):
                scores_batch(b)
            todo_trans.append((b0, nb))
        for b0, nb in chunks[2:]:
            load_part(b0, nb)
            # transposes of an older chunk run while this chunk's DMA is in
            # flight (they sit ahead of its scores in the in-order PE queue)
            while len(todo_trans) > 1:
                tb0, tnb = todo_trans.pop(0)
                for b in range(tb0, tb0 + tnb):
                    trans_batch(b)
            # stages first: a finished group's scores PSUM bank is copied out
            # (freeing the single scg slot) before the next group claims it
            run_stages(b0)
            for b in range(b0, b0 + nb):
                scores_batch(b)
            todo_trans.append((b0, nb))
        run_stages(BL)
        for tb0, tnb in todo_trans:
            for b in range(tb0, tb0 + tnb):
                trans_batch(b)

        # tail, in readiness order
        def flush(si, upto):
            fn = stages[si][0]
            while nstage[si] < upto:
                fn(nstage[si])
                nstage[si] += 1

        flush(0, NG)
        flush(1, NG)
        flush(4, NG - 1)
        flush(2, NG)
        flush(3, NG)
        flush(4, NG)

    nc.compile()
    return nc


LAST_RESULT = None


def kernel(P, id, Wq, bq, Wk, bk, Wv, bv):
    global LAST_RESULT
    from concourse.bass_utils import run_bass_kernel_spmd

    P = np.asarray(P, dtype=np.float32)
    idv = np.asarray(id).astype(np.int32)
    Wq = np.asarray(Wq, dtype=np.float32)
    Wk = np.asarray(Wk, dtype=np.float32)
    Wv = np.asarray(Wv, dtype=np.float32)
    bq = np.asarray(bq, dtype=np.float32)
    bv = np.asarray(bv, dtype=np.float32)

    if "nc" not in _CACHE:
        _CACHE["nc"] = _build()
    nc = _CACHE["nc"]

    # fold the Q and K projections into one matrix (host-side weight prep):
    # qk = Wk^T (Wq p + bq) = M p + v;  lhsT layout wants M^T = Wq^T Wk.
    mt = np.ascontiguousarray((Wq.T @ Wk).astype(np.float32))
    v = np.ascontiguousarray((Wk.T @ bq).astype(np.float32))
    w = np.ascontiguousarray(np.stack([mt, Wv.T]))
    bb = np.ascontiguousarray(np.stack([v, bv]))

    in_maps = []
    for c in range(NCORES):
        sl = slice(c * BL, (c + 1) * BL)
        Pc = P[sl]
        in_maps.append(
            {
                "pt": np.ascontiguousarray(
                    Pc.transpose(0, 2, 1).astype(np.float16)
                ),
                "psel": np.ascontiguousarray(Pc[np.arange(BL), idv[sl]]),
                "w": w,
                "b": bb,
                "bvn": np.ascontiguousarray(np.tile(bv, (G, 1))),
            }
        )

    res = run_bass_kernel_spmd(nc, in_maps, core_ids=list(range(NCORES)))
    LAST_RESULT = res
    out = np.concatenate([r["out"] for r in res.results], axis=0)
    return out
